# revision 1
# baseline (speedup 1.0000x reference)
"""Causal self-attention (B=4, T=2048, C=1024, H=16) on 8 Trainium2 NeuronCores.

Sharding: core = (b, g) with b = core//2 (batch), g = core%2 (head group of 8
heads / 512 features).  Each core computes its batch's attention for its 8
heads plus the partial output projection for its feature slice; the host sums
the two partials per batch and adds the projection bias.

Per-core kernel (all shapes hardcoded).  Inputs are pre-swizzled on the host
so every input DMA reads sequential DRAM, and the first-matmul gate (x block
0 + Wq half) is split one-piece-per-DMA-queue (sync/scalar HWDGE + gpsimd
SWDGE, sized by measured queue bandwidth).

Attention inner loop, per (i-block, head-pair), per 128-key tile jt:
  - ONE PSUM tile [128, 1024] holds both heads' scores for that key tile.
    The two QK^T matmuls (K=64 contraction) target PE row groups 0-63 and
    64-127, become ready together (same tile), are emitted adjacently at
    high priority, so they issue back-to-back and run CONCURRENTLY in the
    PE array (~2x score speedup) and are never queued behind QKV/AV/proj
    matmuls (which would starve the scalar engine's exp chain - the
    original critical path).
  - ONE exp ACTIVATE covers the whole [128, 1024] tile (halves the per-
    instruction ACT overhead); diagonal tiles with o >= 256 dead columns
    split into two truncated ACTIVATEs instead.
  - AV multiplies [V_h | ones64].T @ E so PSUM rows 64:128 hold the softmax
    denominator (free: matmul time is set by the moving dim only).
Blocks above the causal diagonal are skipped entirely; straddling 128x128
blocks get a triangular bf16 mask on gpsimd (keeps DVE free for the bias
adds that release the qkv PSUM slots).  All flexible PE work is zippered
into the attention windows weighted toward late blocks: next block's QKV
during blocks 0-2, ALL projections + block 3's V during block 3 (whose exp
chain is longest but has no QKV left to overlap).  PSUM banks: qkv+proj
share a 2-buf pool, scores 2x[128,1024], AV accumulators 2.
"""

import sys

if "/opt/trn_rl_repo" not in sys.path:
    sys.path.insert(0, "/opt/trn_rl_repo")

import numpy as np

B, T, C, H = 4, 2048, 1024, 16
D = C // H          # 64 head dim
GH = H // 2         # 8 heads per core
CG = C // 2         # 512 features per head group
P = 128             # partitions
NBLK = 512          # free-dim block (t-block / i-block)
N_CORES = 8

_CACHE = {}
RUN_KWARGS = {}     # test harness can set {"trace": True, ...}
LAST_RESULT = [None]

PRIO = 1_000_000    # priority offset for score matmuls


def _build_nc(t=T):
    import concourse.mybir as mybir
    from concourse import bacc
    from concourse.tile import TileContext
    from contextlib import ExitStack

    f32 = mybir.dt.float32
    bf16 = mybir.dt.bfloat16
    Exp = mybir.ActivationFunctionType.Exp

    nt = t // P            # t-tiles
    nib = t // NBLK        # i-blocks / t-blocks
    ck = C // P            # 8 contraction tiles over C
    nm = CG // P           # 4 c'-tiles per group
    blk_t = NBLK // P      # 4 t-tiles per block

    nc = bacc.Bacc("TRN2", target_bir_lowering=False, num_devices=N_CORES)

    # host-preswizzled inputs: every DMA below reads SEQUENTIAL dram.
    # xh row-slab nb = x t-block nb as [p][k][t']; wq/wk row-slab mg = two
    # m-tiles as [p][mh][k][c'].
    xh = nc.dram_tensor("xh", (nib * P, ck * NBLK), bf16, kind="ExternalInput")
    wqh = nc.dram_tensor("wqh", (2 * P, 2 * ck * P), bf16, kind="ExternalInput")
    wkh = nc.dram_tensor("wkh", (2 * P, 2 * ck * P), bf16, kind="ExternalInput")
    wvh = nc.dram_tensor("wvh", (P, ck * CG), bf16, kind="ExternalInput")
    wph = nc.dram_tensor("wph", (P, nm * C), bf16, kind="ExternalInput")
    bqh = nc.dram_tensor("bqh", (P, nm), f32, kind="ExternalInput")
    bkh = nc.dram_tensor("bkh", (P, nm), f32, kind="ExternalInput")
    bvh = nc.dram_tensor("bvh", (1, CG), f32, kind="ExternalInput")
    out = nc.dram_tensor("out", (t, C), f32, kind="ExternalOutput")

    with TileContext(nc) as tc, ExitStack() as es:
        pp = es.enter_context(tc.tile_pool(name="persist", bufs=1))
        epool = es.enter_context(tc.tile_pool(name="e", bufs=12))
        ytpool = es.enter_context(tc.tile_pool(name="yt", bufs=14))
        opool = es.enter_context(tc.tile_pool(name="osb", bufs=8))
        npool = es.enter_context(tc.tile_pool(name="nrm", bufs=4))
        qkvpool = es.enter_context(tc.tile_pool(name="qkv_ps", bufs=2, space="PSUM"))
        stpool = es.enter_context(tc.tile_pool(name="st_ps", bufs=2, space="PSUM"))
        avpool = es.enter_context(tc.tile_pool(name="av_ps", bufs=2, space="PSUM"))
        pjpool = qkvpool

        # ---- all of x resident.  DMA schedule (2 HWDGE queues, by deadline):
        # sync:   x0 rows 0-63, wk mg0, wk mg1, x1, x2, x3
        # scalar: x0 rows 64-127, wq mg0, wq mg1, wv, wp (emitted later)
        # gpsimd: bq, bk, bv (tiny; SWDGE is slow for bulk)
        x_all = pp.tile([P, nib * ck * NBLK], bf16, tag="x_all", name="x_all")
        x_sb = {(nb, k): x_all[:, (nb * ck + k) * NBLK:(nb * ck + k + 1) * NBLK]
                for nb in range(nib) for k in range(ck)}
        # gate for the first matmul = x blk0 + wq mg0: one piece per queue,
        # sized by measured queue speed (sync ~123GB/s, swdge ~40GB/s)
        XS = 96
        nc.sync.dma_start(out=x_all[0:XS, 0:ck * NBLK], in_=xh[0:XS, :])
        nc.gpsimd.dma_start(out=x_all[XS:P, 0:ck * NBLK], in_=xh[XS:P, :])

        # tri[p, y] = 1 if y >= p else 0 -- built right after the x0b issue
        # on gpsimd (no DMA deps) so the HAM warmup matmuls are ready while
        # the input DMAs are still in flight.
        tri = pp.tile([P, P], bf16, tag="tri", name="tri")
        nc.gpsimd.memset(tri, 1.0)
        nc.gpsimd.affine_select(
            out=tri, in_=tri, compare_op=mybir.AluOpType.is_ge,
            fill=0.0, base=0, pattern=[[1, P]], channel_multiplier=-1,
        )
        # ~4us of dummy matmuls flip the PE clock gate to 8/8 before the
        # first real matmul (otherwise the first ~6us of QKV run at 1.2GHz)
        warm = qkvpool.tile([P, P], f32, tag="ps", name="warmup")
        for w_ in range(55):
            nc.tensor.matmul(warm, tri, tri, start=(w_ == 0), stop=(w_ == 54))

        wq_all = pp.tile([P, ck * CG], bf16, tag="wq_all", name="wq_all")
        wk_all = pp.tile([P, ck * CG], bf16, tag="wk_all", name="wk_all")
        nc.scalar.dma_start(out=wq_all[:, 0:2 * ck * P], in_=wqh[0:P, :])
        nc.scalar.dma_start(out=wq_all[:, 2 * ck * P:], in_=wqh[P:2 * P, :])
        nc.sync.dma_start(out=wk_all[:, 0:2 * ck * P], in_=wkh[0:P, :])
        nc.sync.dma_start(out=wk_all[:, 2 * ck * P:], in_=wkh[P:2 * P, :])
        # [mg][mh][k][c'] sbuf layout: (m, k) slice for the QK matmul lhsT
        wq_mk = lambda m, k: wq_all[:, ((m // 2) * 2 * ck + (m % 2) * ck + k) * P:
                                    ((m // 2) * 2 * ck + (m % 2) * ck + k + 1) * P]
        wk_mk = lambda m, k: wk_all[:, ((m // 2) * 2 * ck + (m % 2) * ck + k) * P:
                                    ((m // 2) * 2 * ck + (m % 2) * ck + k + 1) * P]

        bq_sb = pp.tile([P, nm], f32, tag="bq_sb", name="bq_sb")
        nc.gpsimd.dma_start(out=bq_sb, in_=bqh[:, :])
        bk_sb = pp.tile([P, nm], f32, tag="bk_sb", name="bk_sb")
        nc.gpsimd.dma_start(out=bk_sb, in_=bkh[:, :])
        bv_row = pp.tile([1, CG], f32, tag="bv_row", name="bv_row")
        nc.gpsimd.dma_start(out=bv_row, in_=bvh[:, :])
        wv_all = pp.tile([P, ck * CG], bf16, tag="wv_all", name="wv_all")
        HP = P // 2
        nc.gpsimd.dma_start(out=wv_all[0:HP, :], in_=wvh[0:HP, :])
        nc.scalar.dma_start(out=wv_all[HP:P, :], in_=wvh[HP:P, :])
        wv_sb = [wv_all[:, k * CG:(k + 1) * CG] for k in range(ck)]
        for _nb in range(1, nib):
            nc.sync.dma_start(
                out=x_all[:, _nb * ck * NBLK:(_nb + 1) * ck * NBLK],
                in_=xh[_nb * P:(_nb + 1) * P, :])

        bv_bc = pp.tile([P, CG], f32, tag="bv_bc", name="bv_bc")
        nc.gpsimd.partition_broadcast(bv_bc, bv_row)

        qt_sb = {}   # (m, nb) -> (128, 512) bf16 tile of Q^T
        kt_sb = {}
        v_sb = []    # per t-tile (128, 8*128) bf16: per head 64 V cols + 64 ones
        wp_sb = []
        yt_hist = {}

        def qkv_pieces(nb):
            """Return 6 closures: [Q mg0, Q mg1, K mg0, K mg1, V ig0, V ig1]."""
            xts = [x_sb[(nb, k)] for k in range(ck)]

            def qk_piece(which, mg):
                w_mk = wq_mk if which == "q" else wk_mk
                bias = bq_sb if which == "q" else bk_sb
                tgt = qt_sb if which == "q" else kt_sb

                def run():
                    for i in range(2):
                        m = 2 * mg + i
                        ps = qkvpool.tile([P, NBLK], f32, tag="ps", name=f"ps{which}{nb}_{mg}_{i}")
                        for k in range(ck):
                            nc.tensor.matmul(ps, w_mk(m, k), xts[k],
                                             start=(k == 0), stop=(k == ck - 1))
                        tl = pp.tile([P, NBLK], bf16, tag=f"{which}t{m}_{nb}",
                                     name=f"{which}t{m}_{nb}")
                        nc.vector.tensor_scalar_add(tl, ps, bias[:, m:m + 1])
                        tgt[(m, nb)] = tl
                return run

            def v_piece(ig):
                def run():
                    for i in range(2):
                        ps = qkvpool.tile([P, NBLK], f32, tag="ps", name=f"psv{nb}_{ig}_{i}")
                        for k in range(ck):
                            nc.tensor.matmul(ps,
                                             xts[k][:, (2 * ig + i) * P:(2 * ig + i + 1) * P],
                                             wv_sb[k], start=(k == 0), stop=(k == ck - 1))
                        tt = nb * blk_t + 2 * ig + i
                        vt = pp.tile([P, GH * 2 * D], bf16, tag=f"v{tt}", name=f"v{tt}")
                        v3 = vt.rearrange("p (g d) -> p g d", d=2 * D)
                        nc.vector.tensor_add(
                            v3[:, :, 0:D],
                            ps.rearrange("p (h d) -> p h d", d=D),
                            bv_bc.rearrange("p (h d) -> p h d", d=D),
                        )
                        nc.gpsimd.memset(v3[:, :, D:2 * D], 1.0)
                        while len(v_sb) <= tt:
                            v_sb.append(None)
                        v_sb[tt] = vt
                return run

            return [qk_piece("q", 0), qk_piece("q", 1),
                    qk_piece("k", 0), qk_piece("k", 1),
                    v_piece(0), v_piece(1)]

        def emit_attention_pair(ib, pr, last=False):
            """Attention for i-block ib, head pair pr (heads 2pr, 2pr+1)."""
            jt_max = blk_t * (ib + 1)
            ytps = [avpool.tile([P, NBLK], f32, tag="acc", name=f"ytps{ib}_{pr}_{hh}")
                    for hh in range(2)]
            e_tiles = []

            def av(jt):
                o = max(jt * P - ib * NBLK, 0)
                e_j = e_tiles[jt]
                for hh in range(2):
                    h = 2 * pr + hh
                    nc.tensor.matmul(ytps[hh][:, o:NBLK],
                                     v_sb[jt][:, h * 2 * D:(h + 1) * 2 * D],
                                     e_j[:, hh * NBLK + o:(hh + 1) * NBLK],
                                     start=(jt == 0), stop=(jt == jt_max - 1))

            for jt in range(jt_max):
                o = max(jt * P - ib * NBLK, 0)
                o_mm = o if o >= 256 else 0
                st = stpool.tile([P, 2 * NBLK], f32, tag="st", name=f"st{ib}_{pr}_{jt}")
                # the two heads' QK^T target PE row groups 0-63 / 64-127:
                # adjacent + high priority -> they issue back-to-back and
                # overlap in the array, and always beat qkv/av/proj matmuls
                # into the PE queue (the exp chain depends on them).
                jtb, jo = jt // blk_t, (jt % blk_t) * P
                with tc.high_priority(offset=PRIO):
                    for hh in range(2):
                        r = hh * D
                        nc.tensor.matmul(
                            st[:, hh * NBLK + o_mm:(hh + 1) * NBLK],
                            kt_sb[(pr, jtb)][r:r + D, jo:jo + P],
                            qt_sb[(pr, ib)][r:r + D, o_mm:NBLK],
                            start=True, stop=True,
                        )
                e_j = epool.tile([P, 2 * NBLK], bf16, tag="e", name=f"e{ib}_{pr}_{jt}")
                if o_mm:
                    for hh in range(2):
                        nc.scalar.activation(e_j[:, hh * NBLK + o:(hh + 1) * NBLK],
                                             st[:, hh * NBLK + o:(hh + 1) * NBLK],
                                             Exp, scale=0.125)
                else:
                    nc.scalar.activation(e_j, st, Exp, scale=0.125)
                if jt >= jt_max - blk_t:  # straddles the causal diagonal
                    for hh in range(2):
                        # on gpsimd: keeps DVE free for the bias-adds that
                        # release the qkv PSUM slots
                        nc.gpsimd.tensor_mul(e_j[:, hh * NBLK + o:hh * NBLK + o + P],
                                             e_j[:, hh * NBLK + o:hh * NBLK + o + P],
                                             tri)
                e_tiles.append(e_j)
                if jt >= 2:
                    av(jt - 2)
            av(jt_max - 2)
            av(jt_max - 1)

            yt_cur = ytpool.tile([P, NBLK], bf16, tag="yt", name=f"yt{ib}_{pr}")
            from contextlib import nullcontext
            with tc.high_priority(offset=PRIO // 2) if last else nullcontext():
                for hh in range(2):
                    zsb = npool.tile([D, NBLK], f32, tag="zsb", name=f"z{ib}_{pr}_{hh}")
                    if last:  # the exp chain is over; use the idle scalar engine
                        nc.scalar.copy(out=zsb, in_=ytps[hh][D:2 * D, :])
                    else:
                        nc.vector.tensor_copy(out=zsb, in_=ytps[hh][D:2 * D, :])
                    recip = npool.tile([D, NBLK], f32, tag="recip", name=f"rc{ib}_{pr}_{hh}")
                    nc.vector.reciprocal_approx_fast(out=recip, in_=zsb)
                    nc.vector.tensor_mul(yt_cur[hh * D:(hh + 1) * D, :], ytps[hh][0:D, :], recip)
            yt_hist.setdefault(ib, []).append(yt_cur)

        def emit_proj(ib, quarter=None, tail=False):
            yts = yt_hist[ib]
            rng = range(blk_t) if quarter is None else [quarter]
            for i in rng:
                tt = ib * blk_t + i
                for cb in range(C // NBLK):
                    pj = pjpool.tile([P, NBLK], f32, tag="ps", name=f"pj{tt}_{cb}")
                    for p_ in range(nm):
                        nc.tensor.matmul(pj, yts[p_][:, i * P:(i + 1) * P],
                                         wp_sb[p_][:, cb * NBLK:(cb + 1) * NBLK],
                                         start=(p_ == 0), stop=(p_ == nm - 1))
                    ot = opool.tile([P, NBLK], f32, tag="osb", name=f"ot{tt}_{cb}")
                    # at the tail ACT is idle: split the PSUM drains across
                    # both engines so the last copies don't serialize on DVE
                    if tail and cb == 1:
                        nc.scalar.copy(out=ot, in_=pj)
                    else:
                        nc.vector.tensor_copy(out=ot, in_=pj)
                    # sync HWDGE queue is idle after the input loads; the
                    # last stores split across sync+scalar (exp is over, the
                    # scalar queue is idle at the tail)
                    eng = nc.scalar if (tail and cb == 1) else nc.sync
                    eng.dma_start(out=out[tt * P:(tt + 1) * P, cb * NBLK:(cb + 1) * NBLK],
                                  in_=ot)

        for pc in qkv_pieces(0):
            pc()
        wp_all = pp.tile([P, nm * C], bf16, tag="wp_all", name="wp_all")
        nc.scalar.dma_start(out=wp_all, in_=wph[:, :])
        wp_sb.extend(wp_all[:, p_ * C:(p_ + 1) * C] for p_ in range(nm))

        # Zipper: all flexible PE work (next block's QKV, all projections) is
        # placed to fill the attention windows, weighted toward the late
        # blocks whose exp chains are longest (block ib has 16(ib+1) exp
        # tiles but progressively less QKV left to overlap).
        p1 = qkv_pieces(1)
        p2 = qkv_pieces(2)
        p3 = qkv_pieces(3)
        proj = lambda ib, q: (lambda: emit_proj(ib, quarter=q))
        sched = {
            (0, 0): p1[0:2], (0, 1): p1[2:4], (0, 2): p1[4:6], (0, 3): [],
            (1, 0): p2[0:2], (1, 1): p2[2:4], (1, 2): p2[4:6], (1, 3): [],
            (2, 0): p3[0:2] + [proj(0, 0)], (2, 1): p3[2:4] + [proj(0, 1)],
            (2, 2): [proj(0, 2)], (2, 3): [proj(0, 3)],
            (3, 0): p3[4:6] + [proj(1, 0)], (3, 1): [proj(1, 1), proj(1, 2)],
            (3, 2): [proj(1, 3), proj(2, 0), proj(2, 1)],
            (3, 3): [proj(2, 2)],
        }
        for blk in range(nib):
            for pr in range(GH // 2):
                for pc in sched[(blk, pr)]:
                    pc()
                emit_attention_pair(blk, pr, last=(blk == nib - 1 and pr == GH // 2 - 1))
        # held back to fill the PE while the last pair normalizes (also keeps
        # the HAM window busy so the tail projection doesn't run re-throttled)
        emit_proj(2, quarter=3)
        emit_proj(nib - 1, tail=True)

    nc.compile()
    return nc


def _get_nc(t=T):
    if t not in _CACHE:
        _CACHE[t] = _build_nc(t)
    return _CACHE[t]


def kernel(x, Wq, bq, Wk, bk, Wv, bv, Wp, bp):
    import ml_dtypes
    from concourse import bass_utils

    x = np.asarray(x, dtype=np.float32)
    Wq = np.asarray(Wq, dtype=np.float32)
    Wk = np.asarray(Wk, dtype=np.float32)
    Wv = np.asarray(Wv, dtype=np.float32)
    Wp = np.asarray(Wp, dtype=np.float32)
    bq = np.asarray(bq, dtype=np.float32)
    bk = np.asarray(bk, dtype=np.float32)
    bv = np.asarray(bv, dtype=np.float32)
    bp = np.asarray(bp, dtype=np.float32)

    nc = _get_nc()
    bf = ml_dtypes.bfloat16

    nib, ck, nm = T // NBLK, C // P, CG // P

    def swizzle_x(xb):
        # x[b].T is [C, T] = [(k p), (nb t')] -> [nb, p, k, t'] contiguous
        return np.ascontiguousarray(
            xb.T.reshape(ck, P, nib, NBLK).transpose(2, 1, 0, 3).reshape(nib * P, -1)
        ).astype(bf)

    def swizzle_w(w):
        # W[gs].T is [C, CG] = [(k p), c] -> [p, k, c] contiguous
        return np.ascontiguousarray(
            w.reshape(ck, P, CG).transpose(1, 0, 2).reshape(P, -1)
        ).astype(bf)

    def swizzle_w_mk(w):
        # W[gs].T is [C, CG] = [(k p), (mg mh c')] -> [mg, p, mh, k, c']
        return np.ascontiguousarray(
            w.reshape(ck, P, 2, 2, P).transpose(2, 1, 3, 0, 4).reshape(2 * P, -1)
        ).astype(bf)

    def swizzle_wp(wpg):
        # Wp[:, gs].T is [CG, C] = [(a p), c] -> [p, a, c] contiguous
        return np.ascontiguousarray(
            wpg.reshape(nm, P, C).transpose(1, 0, 2).reshape(P, -1)
        ).astype(bf)

    in_maps = []
    for core in range(N_CORES):
        b, g = core // 2, core % 2
        gs = slice(g * CG, (g + 1) * CG)
        in_maps.append({
            "xh": swizzle_x(x[b]),
            "wqh": swizzle_w_mk(Wq[gs, :].T),
            "wkh": swizzle_w_mk(Wk[gs, :].T),
            "wvh": swizzle_w(Wv[gs, :].T),
            "wph": swizzle_wp(Wp[:, gs].T),
            "bqh": np.ascontiguousarray(bq[gs].reshape(CG // P, P).T),
            "bkh": np.ascontiguousarray(bk[gs].reshape(CG // P, P).T),
            "bvh": bv[gs].reshape(1, CG),
        })

    res = bass_utils.run_bass_kernel_spmd(nc, in_maps, core_ids=list(range(N_CORES)),
                                          **RUN_KWARGS)
    LAST_RESULT[0] = res
    y = np.empty((B, T, C), dtype=np.float32)
    for b in range(B):
        y[b] = res.results[2 * b]["out"] + res.results[2 * b + 1]["out"] + bp
    return y

